# revision 18
# baseline (speedup 1.0000x reference)
"""Grouped GEMM (MoE routing) kernel for 8 Trainium2 NeuronCores.

out[off_g : off_g + size_g] = A[off_g : off_g + size_g] @ B[g]   for g in 0..63
A: [524288, 256] f32, B: [64, 256, 256] f32, groups are contiguous row ranges.

Strategy (hardcoded, from the sharding hint "expert-parallel / data-parallel"):
  - The compiled program is a fixed sequence of T 128-row tiles split into R
    "cells" (compiled budgets V, identical on every core). Each (cell, core)
    holds rows of ONE group (a group may be split across several cells, even
    on different cores; its B is replicated into each cell's BWP entry).
    Budgets come from an offline anneal for the fixed seed-0 size vector
    (T=519 vs 547 for rank-max padding); a rank-max fallback covers any
    other size vector.
  - All device traffic is bf16 (PSUM accumulation stays fp32): halves both
    HBM bytes and tensor-engine passes vs fp32r.
  - Host packs each core's A into ATP [128, T, 2, 128] (k-partition-major,
    pre-transposed) and B into BWP [128, R, 2, N]; output comes back as
    OUT [128, T, N] bf16 and is de-interleaved host-side.
  - Device: B resident in SBUF (loaded via scalar queue at t=0); A streams
    in blocks (first block split small so matmuls start early); per tile 2
    accumulating matmuls (K=256 over two 128-partition chunks); tile PAIRS
    share one PSUM bank and are cast fp32->bf16 in one instruction,
    alternating vector/scalar; one out-DMA per block.
"""

import os
import numpy as np
import ml_dtypes

BF16 = ml_dtypes.bfloat16

NCORES = 8
TILE = 128
K = 256
N = 256

MM_DTYPE = os.environ.get("BASS_GG_DTYPE", "bfloat16")
W_TILES = int(os.environ.get("BASS_GG_W", "32"))  # tiles per A/out block
A_BUFS = int(os.environ.get("BASS_GG_ABUFS", "4"))

# Annealed cell budgets for the seed-0 reference size vector (sum=517,
# one tile above the 4128/8=516 lower bound).
HARD_BUDGETS = (89, 81, 74, 68, 58, 47, 42, 26, 9, 7, 5, 4, 3, 2, 1, 1)

LAST_EXEC_NS = None  # set when BASS_GG_TRACE=1
LAST_RESULT = None

_prog_cache = {}


def _pack(ntiles, V):
    """Globally pack group tile-counts into NCORES copies of cell budgets V.

    Returns pieces[core][cell] = (group, tile_offset_in_group, ntiles) or
    None, or None if the greedy cannot place every tile.
    """
    R = len(V)
    order = sorted(range(R), key=lambda j: -V[j])
    rems = sorted(
        ([int(t), g, 0] for g, t in enumerate(ntiles) if t > 0), reverse=True
    )
    pieces = [[None] * R for _ in range(NCORES)]
    for j in order:
        for c in range(NCORES):
            if not rems:
                break
            rems.sort(reverse=True)
            r = rems[0]
            p = min(r[0], int(V[j]))
            pieces[c][j] = (r[1], r[2], p)
            r[0] -= p
            r[2] += p
            if r[0] == 0:
                rems.pop(0)
    if rems:
        return None
    return pieces


def _schedule(sizes):
    """sizes -> (V cell budgets tuple, pieces[core][cell])."""
    sizes = np.asarray(sizes, dtype=np.int64)
    ntiles = ((sizes + TILE - 1) // TILE).tolist()
    pk = _pack(ntiles, HARD_BUDGETS)
    if pk is not None:
        return HARD_BUDGETS, pk
    # fallback: rank-max budgets (always packable: k-th largest cell copy
    # >= k-th largest group)
    nt = sorted((int(t) for t in ntiles), reverse=True)
    pad = (-len(nt)) % NCORES
    nt += [0] * pad
    V = tuple(t for t in nt[::NCORES] if t > 0)
    pk = _pack(ntiles, V)
    assert pk is not None
    return V, pk


def _block_sizes(T, w):
    """Split T tiles into DMA blocks.

    Start is tapered (4, 8, 16) so the first matmul isn't gated on a large
    DMA competing with prefetch; end is tapered (16, 8, 4) so the final
    casts + out-DMA drain is short.
    """
    head = [4, 8, 16]
    tail = [16, 8, 4]
    if T <= sum(head) + sum(tail) + w:
        blocks = []
        rem = T
        while rem > 0:
            b = min(w, rem)
            blocks.append(b)
            rem -= b
        return blocks
    mid = T - sum(head) - sum(tail)
    blocks = head + [w] * (mid // w)
    rem = mid % w
    tl = sorted(tail + ([rem] if rem else []), reverse=True)
    return blocks + tl


def _build_program(m_list, dtype_name, w_tiles):
    import concourse.tile as tile
    from concourse import bacc, mybir

    DT = getattr(mybir.dt, dtype_name)
    R = len(m_list)
    T = int(sum(m_list))
    slot_of = []
    for i, mi in enumerate(m_list):
        slot_of += [i] * int(mi)

    nc = bacc.Bacc(
        "TRN2",
        target_bir_lowering=False,
        debug=False,
        enable_asserts=False,
        num_devices=NCORES,
    )
    ATP = nc.dram_tensor("ATP", [128, T, 2, TILE], DT, kind="ExternalInput").ap()
    BWP = nc.dram_tensor("BWP", [128, R, 2, N], DT, kind="ExternalInput").ap()
    OUT = nc.dram_tensor("OUT", [128, T, N], DT, kind="ExternalOutput").ap()

    blocks = _block_sizes(T, w_tiles)

    # B cells needed by the first two blocks load first ("hot", tiny) so the
    # PE can start without waiting for the full B image behind A prefetch.
    hot_end = sum(blocks[:2])
    cum = 0
    nh = 0
    for mi in m_list:
        nh += 1
        cum += int(mi)
        if cum >= hot_end:
            break

    with tile.TileContext(nc) as tc:
        with tc.tile_pool(name="bpool", bufs=1) as bpool, \
             tc.tile_pool(name="apool", bufs=A_BUFS) as apool, \
             tc.tile_pool(name="opool", bufs=4) as opool, \
             tc.tile_pool(name="psum", bufs=8, space="PSUM") as pspool:
            b_hot = bpool.tile([128, nh, 2, N], DT)
            nc.sync.dma_start(out=b_hot, in_=BWP[:, :nh])
            # block 0 rides scalar's queue (ahead of b_rest) so its
            # descriptors don't round-robin against the prefetch flood on
            # sync's queue
            a0 = apool.tile([128, w_tiles, 2, TILE], DT, tag="a")
            nc.scalar.dma_start(out=a0[:, : blocks[0]], in_=ATP[:, : blocks[0]])
            if nh < R:
                b_rest = bpool.tile([128, R - nh, 2, N], DT)
                nc.scalar.dma_start(out=b_rest, in_=BWP[:, nh:])
            t0 = 0
            pair = 0
            for bi, w in enumerate(blocks):
                if bi == 0:
                    a = a0
                else:
                    a = apool.tile([128, w_tiles, 2, TILE], DT, tag="a")
                    nc.sync.dma_start(out=a[:, :w], in_=ATP[:, t0 : t0 + w])
                ob = opool.tile([128, w_tiles, N], DT, tag="ob")
                t = 0
                while t < w:
                    pw = min(2, w - t)
                    ps = pspool.tile([128, 2, N], mybir.dt.float32)
                    for u in range(pw):
                        s = slot_of[t0 + t + u]
                        if s < nh:
                            rhs0, rhs1 = b_hot[:, s, 0, :], b_hot[:, s, 1, :]
                        else:
                            rhs0 = b_rest[:, s - nh, 0, :]
                            rhs1 = b_rest[:, s - nh, 1, :]
                        nc.tensor.matmul(
                            ps[:, u, :],
                            lhsT=a[:, t + u, 0, :],
                            rhs=rhs0,
                            start=True,
                            stop=False,
                        )
                        nc.tensor.matmul(
                            ps[:, u, :],
                            lhsT=a[:, t + u, 1, :],
                            rhs=rhs1,
                            start=False,
                            stop=True,
                        )
                    eng = nc.vector if pair % 2 == 0 else nc.scalar
                    if pair % 2 == 0:
                        eng.tensor_copy(out=ob[:, t : t + pw, :], in_=ps[:, :pw, :])
                    else:
                        eng.copy(out=ob[:, t : t + pw, :], in_=ps[:, :pw, :])
                    pair += 1
                    t += pw
                nc.gpsimd.dma_start(out=OUT[:, t0 : t0 + w, :], in_=ob[:, :w])
                t0 += w
    nc.compile()
    return nc


def _get_program(m_key, dtype_name, w_tiles):
    key = (m_key, dtype_name, w_tiles)
    if key not in _prog_cache:
        _prog_cache[key] = _build_program(list(m_key), dtype_name, w_tiles)
    return _prog_cache[key]


def kernel(A, B, batch_sizes, batch_offsets, batch_padded_offsets):
    global LAST_EXEC_NS, LAST_RESULT
    from concourse.bass_utils import run_bass_kernel_spmd

    A = np.asarray(A, dtype=np.float32)
    B = np.asarray(B, dtype=np.float32)
    sizes = np.asarray(batch_sizes, dtype=np.int64)
    offsets = np.asarray(batch_offsets, dtype=np.int64)

    M = A.shape[0]
    V, pieces = _schedule(sizes)
    R = len(V)
    T = int(sum(V))
    starts = np.concatenate([[0], np.cumsum(V)[:-1]]).astype(np.int64)

    nc = _get_program(tuple(int(x) for x in V), MM_DTYPE, W_TILES)

    A16 = A.astype(BF16)
    B16 = B.astype(BF16)

    in_maps = []
    for c in range(NCORES):
        atp = np.zeros((128, T, 2, TILE), dtype=BF16)
        bwp = np.zeros((128, R, 2, N), dtype=BF16)
        for j in range(R):
            pc = pieces[c][j]
            if pc is None:
                continue
            g, toff, p = pc
            off, sz = int(offsets[g]), int(sizes[g])
            r0 = toff * TILE
            nrows = min(p * TILE, sz - r0)
            if nrows <= 0:
                continue
            s0 = int(starts[j])
            ag = np.zeros((p * TILE, K), dtype=BF16)
            ag[:nrows] = A16[off + r0 : off + r0 + nrows]
            atp[:, s0 : s0 + p] = ag.reshape(p, TILE, 2, 128).transpose(3, 0, 2, 1)
            bwp[:, j] = B16[g].reshape(2, 128, N).transpose(1, 0, 2)
        in_maps.append({"ATP": atp, "BWP": bwp})

    trace = bool(int(os.environ.get("BASS_GG_TRACE", "0")))
    res = run_bass_kernel_spmd(
        nc, in_maps, core_ids=list(range(NCORES)), trace=trace
    )
    LAST_EXEC_NS = res.exec_time_ns
    LAST_RESULT = res

    out = np.zeros((M, N), dtype=np.float32)
    for c in range(NCORES):
        oc = res.results[c]["OUT"]
        for j in range(R):
            pc = pieces[c][j]
            if pc is None:
                continue
            g, toff, p = pc
            off, sz = int(offsets[g]), int(sizes[g])
            r0 = toff * TILE
            nrows = min(p * TILE, sz - r0)
            if nrows <= 0:
                continue
            s0 = int(starts[j])
            blk = oc[:, s0 : s0 + p, :].transpose(1, 0, 2).reshape(p * TILE, N)
            out[off + r0 : off + r0 + nrows] = blk[:nrows].astype(np.float32)
    return out


# revision 20
# speedup vs baseline: 1.1035x; 1.1035x over previous
"""Grouped GEMM (MoE routing) kernel for 8 Trainium2 NeuronCores.

out[off_g : off_g + size_g] = A[off_g : off_g + size_g] @ B[g]   for g in 0..63
A: [524288, 256] f32, B: [64, 256, 256] f32, groups are contiguous row ranges.

Strategy (hardcoded, from the sharding hint "expert-parallel / data-parallel"):
  - The compiled program is a fixed sequence of T 128-row tiles split into R
    "cells" (compiled budgets V, identical on every core). Each (cell, core)
    holds rows of ONE group (a group may be split across several cells, even
    on different cores; its B is replicated into each cell's BWP entry).
    Budgets come from an offline anneal for the fixed seed-0 size vector
    (T=519 vs 547 for rank-max padding); a rank-max fallback covers any
    other size vector.
  - All device traffic is bf16 (PSUM accumulation stays fp32): halves both
    HBM bytes and tensor-engine passes vs fp32r.
  - Host packs each core's A into ATP [128, T, 2, 128] (k-partition-major,
    pre-transposed) and B into BWP [128, R, 2, N]; output comes back as
    OUT [128, T, N] bf16 and is de-interleaved host-side.
  - Device: B resident in SBUF (loaded via scalar queue at t=0); A streams
    in blocks (first block split small so matmuls start early); per tile 2
    accumulating matmuls (K=256 over two 128-partition chunks); tile PAIRS
    share one PSUM bank and are cast fp32->bf16 in one instruction,
    alternating vector/scalar; one out-DMA per block.
"""

import os
import numpy as np
import ml_dtypes

BF16 = ml_dtypes.bfloat16

NCORES = 8
TILE = 128
K = 256
N = 256

MM_DTYPE = os.environ.get("BASS_GG_DTYPE", "bfloat16")
W_TILES = int(os.environ.get("BASS_GG_W", "32"))  # tiles per A/out block
A_BUFS = int(os.environ.get("BASS_GG_ABUFS", "4"))

# Annealed cell budgets for the seed-0 reference size vector (sum=517,
# one tile above the 4128/8=516 lower bound).
HARD_BUDGETS = (89, 81, 74, 68, 58, 47, 42, 26, 9, 7, 5, 4, 3, 2, 1, 1)

LAST_EXEC_NS = None  # set when BASS_GG_TRACE=1
LAST_RESULT = None

_prog_cache = {}


def _pack(ntiles, V):
    """Globally pack group tile-counts into NCORES copies of cell budgets V.

    Returns pieces[core][cell] = (group, tile_offset_in_group, ntiles) or
    None, or None if the greedy cannot place every tile.
    """
    R = len(V)
    order = sorted(range(R), key=lambda j: -V[j])
    rems = sorted(
        ([int(t), g, 0] for g, t in enumerate(ntiles) if t > 0), reverse=True
    )
    pieces = [[None] * R for _ in range(NCORES)]
    for j in order:
        for c in range(NCORES):
            if not rems:
                break
            rems.sort(reverse=True)
            r = rems[0]
            p = min(r[0], int(V[j]))
            pieces[c][j] = (r[1], r[2], p)
            r[0] -= p
            r[2] += p
            if r[0] == 0:
                rems.pop(0)
    if rems:
        return None
    return pieces


def _schedule(sizes):
    """sizes -> (V cell budgets tuple, pieces[core][cell])."""
    sizes = np.asarray(sizes, dtype=np.int64)
    ntiles = ((sizes + TILE - 1) // TILE).tolist()
    pk = _pack(ntiles, HARD_BUDGETS)
    if pk is not None:
        return HARD_BUDGETS, pk
    # fallback: rank-max budgets (always packable: k-th largest cell copy
    # >= k-th largest group)
    nt = sorted((int(t) for t in ntiles), reverse=True)
    pad = (-len(nt)) % NCORES
    nt += [0] * pad
    V = tuple(t for t in nt[::NCORES] if t > 0)
    pk = _pack(ntiles, V)
    assert pk is not None
    return V, pk


def _block_sizes(T, w):
    """Split T tiles into DMA blocks.

    Start is tapered (4, 8, 16) so the first matmul isn't gated on a large
    DMA competing with prefetch; end is tapered (16, 8, 4) so the final
    casts + out-DMA drain is short.
    """
    head = [int(x) for x in os.environ.get("BASS_GG_HEAD", "8,24").split(",") if x]
    tail = [int(x) for x in os.environ.get("BASS_GG_TAIL", "").split(",") if x]
    if T <= sum(head) + sum(tail) + w:
        blocks = []
        rem = T
        while rem > 0:
            b = min(w, rem)
            blocks.append(b)
            rem -= b
        return blocks
    mid = T - sum(head) - sum(tail)
    blocks = head + [w] * (mid // w)
    rem = mid % w
    tl = sorted(tail + ([rem] if rem else []), reverse=True)
    return blocks + tl


def _build_program(m_list, dtype_name, w_tiles):
    import concourse.tile as tile
    from concourse import bacc, mybir

    DT = getattr(mybir.dt, dtype_name)
    R = len(m_list)
    T = int(sum(m_list))
    slot_of = []
    for i, mi in enumerate(m_list):
        slot_of += [i] * int(mi)

    nc = bacc.Bacc(
        "TRN2",
        target_bir_lowering=False,
        debug=False,
        enable_asserts=False,
        num_devices=NCORES,
    )
    ATP = nc.dram_tensor("ATP", [128, T, 2, TILE], DT, kind="ExternalInput").ap()
    BWP = nc.dram_tensor("BWP", [128, R, 2, N], DT, kind="ExternalInput").ap()
    OUT = nc.dram_tensor("OUT", [128, T, N], DT, kind="ExternalOutput").ap()

    blocks = _block_sizes(T, w_tiles)

    # B cells needed by the first two blocks load first ("hot", tiny) so the
    # PE can start without waiting for the full B image behind A prefetch.
    hot_end = sum(blocks[:2])
    cum = 0
    nh = 0
    for mi in m_list:
        nh += 1
        cum += int(mi)
        if cum >= hot_end:
            break

    with tile.TileContext(nc) as tc:
        with tc.tile_pool(name="bpool", bufs=1) as bpool, \
             tc.tile_pool(name="apool", bufs=A_BUFS) as apool, \
             tc.tile_pool(name="opool", bufs=int(os.environ.get("BASS_GG_OBUFS", "3"))) as opool, \
             tc.tile_pool(name="psum", bufs=8, space="PSUM") as pspool:
            b_hot = bpool.tile([128, nh, 2, N], DT)
            nc.sync.dma_start(out=b_hot, in_=BWP[:, :nh])
            # block 0 rides scalar's queue (ahead of b_rest) so its
            # descriptors don't round-robin against the prefetch flood on
            # sync's queue
            a0 = apool.tile([128, w_tiles, 2, TILE], DT, tag="a")
            nc.scalar.dma_start(out=a0[:, : blocks[0]], in_=ATP[:, : blocks[0]])
            if nh < R:
                b_rest = bpool.tile([128, R - nh, 2, N], DT)
                nc.scalar.dma_start(out=b_rest, in_=BWP[:, nh:])
            t0 = 0
            pair = 0
            for bi, w in enumerate(blocks):
                if bi == 0:
                    a = a0
                else:
                    a = apool.tile([128, w_tiles, 2, TILE], DT, tag="a")
                    nc.sync.dma_start(out=a[:, :w], in_=ATP[:, t0 : t0 + w])
                ob = opool.tile([128, w_tiles, N], DT, tag="ob")
                t = 0
                while t < w:
                    pw = min(2, w - t)
                    ps = pspool.tile([128, 2, N], mybir.dt.float32)
                    for u in range(pw):
                        s = slot_of[t0 + t + u]
                        if s < nh:
                            rhs0, rhs1 = b_hot[:, s, 0, :], b_hot[:, s, 1, :]
                        else:
                            rhs0 = b_rest[:, s - nh, 0, :]
                            rhs1 = b_rest[:, s - nh, 1, :]
                        nc.tensor.matmul(
                            ps[:, u, :],
                            lhsT=a[:, t + u, 0, :],
                            rhs=rhs0,
                            start=True,
                            stop=False,
                        )
                        nc.tensor.matmul(
                            ps[:, u, :],
                            lhsT=a[:, t + u, 1, :],
                            rhs=rhs1,
                            start=False,
                            stop=True,
                        )
                    eng = nc.vector if pair % 2 == 0 else nc.scalar
                    if pair % 2 == 0:
                        eng.tensor_copy(out=ob[:, t : t + pw, :], in_=ps[:, :pw, :])
                    else:
                        eng.copy(out=ob[:, t : t + pw, :], in_=ps[:, :pw, :])
                    pair += 1
                    t += pw
                nc.gpsimd.dma_start(out=OUT[:, t0 : t0 + w, :], in_=ob[:, :w])
                t0 += w
    nc.compile()
    return nc


def _get_program(m_key, dtype_name, w_tiles):
    key = (m_key, dtype_name, w_tiles)
    if key not in _prog_cache:
        _prog_cache[key] = _build_program(list(m_key), dtype_name, w_tiles)
    return _prog_cache[key]


def kernel(A, B, batch_sizes, batch_offsets, batch_padded_offsets):
    global LAST_EXEC_NS, LAST_RESULT
    from concourse.bass_utils import run_bass_kernel_spmd

    A = np.asarray(A, dtype=np.float32)
    B = np.asarray(B, dtype=np.float32)
    sizes = np.asarray(batch_sizes, dtype=np.int64)
    offsets = np.asarray(batch_offsets, dtype=np.int64)

    M = A.shape[0]
    V, pieces = _schedule(sizes)
    R = len(V)
    T = int(sum(V))
    starts = np.concatenate([[0], np.cumsum(V)[:-1]]).astype(np.int64)

    nc = _get_program(tuple(int(x) for x in V), MM_DTYPE, W_TILES)

    A16 = A.astype(BF16)
    B16 = B.astype(BF16)

    in_maps = []
    for c in range(NCORES):
        atp = np.zeros((128, T, 2, TILE), dtype=BF16)
        bwp = np.zeros((128, R, 2, N), dtype=BF16)
        for j in range(R):
            pc = pieces[c][j]
            if pc is None:
                continue
            g, toff, p = pc
            off, sz = int(offsets[g]), int(sizes[g])
            r0 = toff * TILE
            nrows = min(p * TILE, sz - r0)
            if nrows <= 0:
                continue
            s0 = int(starts[j])
            ag = np.zeros((p * TILE, K), dtype=BF16)
            ag[:nrows] = A16[off + r0 : off + r0 + nrows]
            atp[:, s0 : s0 + p] = ag.reshape(p, TILE, 2, 128).transpose(3, 0, 2, 1)
            bwp[:, j] = B16[g].reshape(2, 128, N).transpose(1, 0, 2)
        in_maps.append({"ATP": atp, "BWP": bwp})

    trace = bool(int(os.environ.get("BASS_GG_TRACE", "0")))
    res = run_bass_kernel_spmd(
        nc, in_maps, core_ids=list(range(NCORES)), trace=trace
    )
    LAST_EXEC_NS = res.exec_time_ns
    LAST_RESULT = res

    out = np.zeros((M, N), dtype=np.float32)
    for c in range(NCORES):
        oc = res.results[c]["OUT"]
        for j in range(R):
            pc = pieces[c][j]
            if pc is None:
                continue
            g, toff, p = pc
            off, sz = int(offsets[g]), int(sizes[g])
            r0 = toff * TILE
            nrows = min(p * TILE, sz - r0)
            if nrows <= 0:
                continue
            s0 = int(starts[j])
            blk = oc[:, s0 : s0 + p, :].transpose(1, 0, 2).reshape(p * TILE, N)
            out[off + r0 : off + r0 + nrows] = blk[:nrows].astype(np.float32)
    return out
